# revision 4
# baseline (speedup 1.0000x reference)
"""Chamfer distance kernel for Trainium2 (8 NeuronCores).

Problem: x, y ~ (B=4, N=8192, 3) fp32. loss = mean_b[ mean_n min_m dist + mean_m min_n dist ].

Strategy:
  - d2[n,m] = ||x_n||^2 + ||y_m||^2 - 2 x.y  computed as a single augmented
    matmul with K=5: lhsT rows = [-2x0,-2x1,-2x2, ||x||^2, 1],
    rhs rows = [y0,y1,y2, 1, ||y||^2].  PSUM holds d2 directly.
  - min over the free axis on DVE; sqrt/means on host (monotone => sqrt after min).
  - 8 cores = (batch b, half h): each core does both directions for its half
    of the rows against the full opposite set => no cross-core reduction.
  - Host does the tiny augmentation ([5,N] builds) and final mean/sqrt.
"""

import sys

sys.path.insert(0, "/opt/trn_rl_repo")

import numpy as np

import concourse.bass as bass
import concourse.bacc as bacc
import concourse.mybir as mybir
from concourse import tile
from concourse.bass_utils import run_bass_kernel_spmd

N_CORES = 8
B, N, M, D = 4, 8192, 8192, 3
HALF = N // 2  # rows per core per direction
NT = HALF // 128  # 32 n-tiles per pass
MCHUNK = 2048  # psum tile free size (4 banks)
NJ = M // MCHUNK  # 4 chunks
F32 = mybir.dt.float32

_NC_CACHE = {}


def build_bass():
    nc = bacc.Bacc(
        "TRN2", target_bir_lowering=False, debug=False, num_devices=N_CORES
    )
    la = nc.dram_tensor("la", [5, HALF], F32, kind="ExternalInput")
    ra = nc.dram_tensor("ra", [5, M], F32, kind="ExternalInput")
    lb = nc.dram_tensor("lb", [5, HALF], F32, kind="ExternalInput")
    rb = nc.dram_tensor("rb", [5, M], F32, kind="ExternalInput")
    out = nc.dram_tensor("out", [128, 2 * NT], F32, kind="ExternalOutput")

    with tile.TileContext(nc) as tc:
        with (
            tc.tile_pool(name="inp", bufs=1) as inp,
            tc.tile_pool(name="psum", bufs=2, space="PSUM") as psum,
            tc.tile_pool(name="part", bufs=2) as partp,
            tc.tile_pool(name="res", bufs=1) as resp,
        ):
            ls_a = inp.tile([5, HALF], F32, tag="la")
            nc.sync.dma_start(ls_a[:], la[:])
            rs_a = inp.tile([5, M], F32, tag="ra")
            nc.sync.dma_start(rs_a[:], ra[:])
            ls_b = inp.tile([5, HALF], F32, tag="lb")
            nc.sync.dma_start(ls_b[:], lb[:])
            rs_b = inp.tile([5, M], F32, tag="rb")
            nc.sync.dma_start(rs_b[:], rb[:])

            out_s = resp.tile([128, 2 * NT], F32)

            for p, (lt, rt) in enumerate([(ls_a, rs_a), (ls_b, rs_b)]):
                for t in range(NT):
                    part = partp.tile([128, NJ], F32)
                    for j in range(NJ):
                        ps = psum.tile([128, MCHUNK], F32)
                        for k in range(MCHUNK // 512):
                            nc.tensor.matmul(
                                ps[:, k * 512 : (k + 1) * 512],
                                lt[:, t * 128 : (t + 1) * 128],
                                rt[:, j * MCHUNK + k * 512 : j * MCHUNK + (k + 1) * 512],
                            )
                        nc.vector.tensor_reduce(
                            part[:, j : j + 1],
                            ps[:],
                            axis=mybir.AxisListType.X,
                            op=mybir.AluOpType.min,
                        )
                    nc.vector.tensor_reduce(
                        out_s[:, p * NT + t : p * NT + t + 1],
                        part[:],
                        axis=mybir.AxisListType.X,
                        op=mybir.AluOpType.min,
                    )
            nc.sync.dma_start(out[:], out_s[:])
    nc.compile()
    return nc


def _lhs_aug(p):
    # p: [n, 3] fp32 -> [5, n]: rows [-2x0, -2x1, -2x2, ||x||^2, 1]
    p2 = np.sum(p * p, axis=-1)
    return np.ascontiguousarray(
        np.stack([-2.0 * p[:, 0], -2.0 * p[:, 1], -2.0 * p[:, 2], p2, np.ones_like(p2)], 0)
    ).astype(np.float32)


def _rhs_aug(p):
    # p: [m, 3] fp32 -> [5, m]: rows [y0, y1, y2, 1, ||y||^2]
    p2 = np.sum(p * p, axis=-1)
    return np.ascontiguousarray(
        np.stack([p[:, 0], p[:, 1], p[:, 2], np.ones_like(p2), p2], 0)
    ).astype(np.float32)


def _run(x, y, trace=False):
    if "nc" not in _NC_CACHE:
        _NC_CACHE["nc"] = build_bass()
    nc = _NC_CACHE["nc"]

    in_maps = []
    for c in range(N_CORES):
        b, h = divmod(c, 2)
        xs = x[b, h * HALF : (h + 1) * HALF]
        ys = y[b, h * HALF : (h + 1) * HALF]
        in_maps.append(
            {
                "la": _lhs_aug(xs),
                "ra": _rhs_aug(y[b]),
                "lb": _lhs_aug(ys),
                "rb": _rhs_aug(x[b]),
            }
        )
    return run_bass_kernel_spmd(nc, in_maps, list(range(N_CORES)), trace=trace)


def kernel(x, y, bidirectional):
    x = np.asarray(x, dtype=np.float32)
    y = np.asarray(y, dtype=np.float32)
    bidir = int(np.asarray(bidirectional))

    res = _run(x, y).results

    term1 = np.zeros(B, dtype=np.float64)
    term2 = np.zeros(B, dtype=np.float64)
    for c in range(N_CORES):
        b, h = divmod(c, 2)
        o = np.asarray(res[c]["out"])
        rma = o[:, :NT].T.reshape(-1)  # d2 row mins, x-half -> over all y
        rmb = o[:, NT:].T.reshape(-1)  # d2 row mins, y-half -> over all x
        term1[b] += np.sqrt(np.maximum(rma, 0.0)).mean() / 2.0
        term2[b] += np.sqrt(np.maximum(rmb, 0.0)).mean() / 2.0

    loss = term1.mean()
    if bidir:
        loss = loss + term2.mean()
    return np.float32(loss)


# revision 8
# speedup vs baseline: 1.5159x; 1.5159x over previous
"""Chamfer distance kernel for Trainium2 (8 NeuronCores).

Problem: x, y ~ (B=4, N=8192, 3) fp32. loss = mean_b[ mean_n min_m dist + mean_m min_n dist ].

Strategy:
  - d2[n,m] = ||x_n||^2 + ||y_m||^2 - 2 x.y  computed as a single augmented
    matmul with K=5: lhsT rows = [-2x0,-2x1,-2x2, ||x||^2, 1],
    rhs rows = [y0,y1,y2, 1, ||y||^2].  PSUM holds d2 directly.
  - min over the free axis on DVE; sqrt/means on host (monotone => sqrt after min).
  - 8 cores = (batch b, half h): each core does both directions for its half
    of the rows against the full opposite set => no cross-core reduction.
  - Host does the tiny augmentation ([5,N] builds) and final mean/sqrt.
"""

import sys

sys.path.insert(0, "/opt/trn_rl_repo")

import numpy as np

import concourse.bass as bass
import concourse.bacc as bacc
import concourse.mybir as mybir
from concourse import tile
from concourse.bass_utils import run_bass_kernel_spmd

N_CORES = 8
B, N, M, D = 4, 8192, 8192, 3
HALF = N // 2  # rows per core per direction
NT = HALF // 128  # 32 n-tiles per pass
MCHUNK = 2048  # psum tile free size (4 banks)
NJ = M // MCHUNK  # 4 chunks
K = 13  # augmented fp16 hi/lo contraction rows
F32 = mybir.dt.float32
F16 = mybir.dt.float16

_NC_CACHE = {}


def build_bass():
    nc = bacc.Bacc(
        "TRN2", target_bir_lowering=False, debug=False, num_devices=N_CORES
    )
    la = nc.dram_tensor("la", [K, HALF], F16, kind="ExternalInput")
    ra = nc.dram_tensor("ra", [K, M], F16, kind="ExternalInput")
    lb = nc.dram_tensor("lb", [K, HALF], F16, kind="ExternalInput")
    rb = nc.dram_tensor("rb", [K, M], F16, kind="ExternalInput")
    out = nc.dram_tensor("out", [128, 2 * NT], F32, kind="ExternalOutput")

    with tile.TileContext(nc) as tc:
        with (
            tc.tile_pool(name="inp", bufs=1) as inp,
            tc.tile_pool(name="psum", bufs=2, space="PSUM") as psum,
            tc.tile_pool(name="part", bufs=2) as partp,
            tc.tile_pool(name="res", bufs=1) as resp,
        ):
            ls_a = inp.tile([K, HALF], F16, tag="la")
            nc.sync.dma_start(ls_a[:], la[:])
            rs_a = inp.tile([K, M], F16, tag="ra")
            nc.sync.dma_start(rs_a[:], ra[:])
            ls_b = inp.tile([K, HALF], F16, tag="lb")
            nc.sync.dma_start(ls_b[:], lb[:])
            rs_b = inp.tile([K, M], F16, tag="rb")
            nc.sync.dma_start(rs_b[:], rb[:])

            out_s = resp.tile([128, 2 * NT], F32)

            for p, (lt, rt) in enumerate([(ls_a, rs_a), (ls_b, rs_b)]):
                for t in range(NT):
                    part = partp.tile([128, NJ], F32)
                    for j in range(NJ):
                        ps = psum.tile([128, MCHUNK], F32)
                        for k in range(MCHUNK // 512):
                            nc.tensor.matmul(
                                ps[:, k * 512 : (k + 1) * 512],
                                lt[:, t * 128 : (t + 1) * 128],
                                rt[:, j * MCHUNK + k * 512 : j * MCHUNK + (k + 1) * 512],
                            )
                        nc.vector.tensor_reduce(
                            part[:, j : j + 1],
                            ps[:],
                            axis=mybir.AxisListType.X,
                            op=mybir.AluOpType.min,
                        )
                    nc.vector.tensor_reduce(
                        out_s[:, p * NT + t : p * NT + t + 1],
                        part[:],
                        axis=mybir.AxisListType.X,
                        op=mybir.AluOpType.min,
                    )
            nc.sync.dma_start(out[:], out_s[:])
    nc.compile()
    return nc


def _split16(v):
    h = v.astype(np.float16)
    l = (v - h.astype(np.float32)).astype(np.float16)
    return h, l


def _lhs_aug(p):
    # p: [n, 3] fp32 -> [13, n] fp16 hi/lo split; pairs with _rhs_aug rows so
    # that sum_k lhs[k,n]*rhs[k,m] = ||x||^2 + ||y||^2 - 2 x.y to ~fp32 accuracy
    p2 = np.sum(p * p, axis=-1)
    c = p.T
    ch, cl = _split16(c)
    p2h, p2l = _split16(p2)
    ones = np.ones_like(p2, dtype=np.float16)
    rows = [
        -2 * ch[0], -2 * ch[1], -2 * ch[2],  # * y_h
        -2 * ch[0], -2 * ch[1], -2 * ch[2],  # * y_l
        -2 * cl[0], -2 * cl[1], -2 * cl[2],  # * y_h
        p2h, p2l, ones, ones,
    ]
    return np.ascontiguousarray(np.stack([r.astype(np.float16) for r in rows], 0))


def _rhs_aug(p):
    p2 = np.sum(p * p, axis=-1)
    c = p.T
    ch, cl = _split16(c)
    p2h, p2l = _split16(p2)
    ones = np.ones_like(p2, dtype=np.float16)
    rows = [
        ch[0], ch[1], ch[2],
        cl[0], cl[1], cl[2],
        ch[0], ch[1], ch[2],
        ones, ones, p2h, p2l,
    ]
    return np.ascontiguousarray(np.stack([r.astype(np.float16) for r in rows], 0))


def _run(x, y, trace=False):
    if "nc" not in _NC_CACHE:
        _NC_CACHE["nc"] = build_bass()
    nc = _NC_CACHE["nc"]

    in_maps = []
    for c in range(N_CORES):
        b, h = divmod(c, 2)
        xs = x[b, h * HALF : (h + 1) * HALF]
        ys = y[b, h * HALF : (h + 1) * HALF]
        in_maps.append(
            {
                "la": _lhs_aug(xs),
                "ra": _rhs_aug(y[b]),
                "lb": _lhs_aug(ys),
                "rb": _rhs_aug(x[b]),
            }
        )
    return run_bass_kernel_spmd(nc, in_maps, list(range(N_CORES)), trace=trace)


def kernel(x, y, bidirectional):
    x = np.asarray(x, dtype=np.float32)
    y = np.asarray(y, dtype=np.float32)
    bidir = int(np.asarray(bidirectional))

    res = _run(x, y).results

    term1 = np.zeros(B, dtype=np.float64)
    term2 = np.zeros(B, dtype=np.float64)
    for c in range(N_CORES):
        b, h = divmod(c, 2)
        o = np.asarray(res[c]["out"])
        rma = o[:, :NT].T.reshape(-1)  # d2 row mins, x-half -> over all y
        rmb = o[:, NT:].T.reshape(-1)  # d2 row mins, y-half -> over all x
        term1[b] += np.sqrt(np.maximum(rma, 0.0)).mean() / 2.0
        term2[b] += np.sqrt(np.maximum(rmb, 0.0)).mean() / 2.0

    loss = term1.mean()
    if bidir:
        loss = loss + term2.mean()
    return np.float32(loss)


# revision 11
# speedup vs baseline: 1.5824x; 1.0439x over previous
"""Chamfer distance kernel for Trainium2 (8 NeuronCores).

Problem: x, y ~ (B=4, N=8192, 3) fp32. loss = mean_b[ mean_n min_m dist + mean_m min_n dist ].

Strategy:
  - d2[n,m] = ||x||^2 + ||y||^2 - 2 x.y as ONE augmented matmul: the fp32
    values are split hi/lo into fp16 (exact products in PSUM fp32), giving
    K=13 fp16 contraction rows -> 4x faster PE than fp32, ~1e-7 accuracy.
  - K=13 <= 32, so 4 independent matmuls are packed into the PE array via
    tile_position row-groups (base partitions 0/32/64/96) -> ~4x concurrency.
  - min over the free axis; sqrt/means on host (monotone => sqrt after min).
  - Two consumer lanes, balanced so DVE and ACT both stay busy:
      direct blocks: DVE reduce_min straight from PSUM (1 elem/lane/cyc)
      ACT blocks:    ScalarE Relu-cast-copies PSUM->SBUF fp16, then DVE
                     tensor_tensor min folds at 2x (fp16 2X_1P) + one reduce
    (tensor_tensor_reduce would be 1 instr/block but crashes this runtime)
  - 8 cores = (batch b, half h): each core does both directions for its half
    of the rows against the full opposite set => no cross-core reduction.
"""

import sys

sys.path.insert(0, "/opt/trn_rl_repo")

import numpy as np

import concourse.bass as bass
import concourse.bacc as bacc
import concourse.mybir as mybir
from concourse import tile
from concourse.bass_utils import run_bass_kernel_spmd

N_CORES = 8
B, N, M, D = 4, 8192, 8192, 3
HALF = N // 2  # rows per core per direction
NT = HALF // 128  # 32 n-tiles per pass
MCHUNK = 2048  # psum tile free size (4 banks)
NJ = M // MCHUNK  # 4 chunks per block
K = 13  # augmented fp16 hi/lo contraction rows
DIRECT_EVERY = 4  # every 4th block bypasses ACT (DVE reduces from PSUM)
BIG = 60000.0  # > max possible d2, fits fp16
F32 = mybir.dt.float32
F16 = mybir.dt.float16

_NC_CACHE = {}


def build_bass():
    nc = bacc.Bacc(
        "TRN2", target_bir_lowering=False, debug=False, num_devices=N_CORES
    )
    la = nc.dram_tensor("la", [K, HALF], F16, kind="ExternalInput")
    ra = nc.dram_tensor("ra", [K, M], F16, kind="ExternalInput")
    lb = nc.dram_tensor("lb", [K, HALF], F16, kind="ExternalInput")
    rb = nc.dram_tensor("rb", [K, M], F16, kind="ExternalInput")
    out = nc.dram_tensor("out", [128, 2 * NT], F32, kind="ExternalOutput")

    with tile.TileContext(nc) as tc:
        with (
            tc.tile_pool(name="inp", bufs=1) as inp,
            tc.tile_pool(name="psum", bufs=2, space="PSUM") as psum,
            tc.tile_pool(name="stg", bufs=2) as stg,
            tc.tile_pool(name="junkp", bufs=1) as junkp,
            tc.tile_pool(name="part", bufs=2) as partp,
            tc.tile_pool(name="res", bufs=1) as resp,
        ):
            # inputs replicated at base partitions 0/32/64/96 for row-group
            # packed matmuls (4 concurrent MMs in the PE array)
            ls_a = inp.tile([128, HALF], F16, tag="la")
            rs_a = inp.tile([128, M], F16, tag="ra")
            ls_b = inp.tile([128, HALF], F16, tag="lb")
            rs_b = inp.tile([128, M], F16, tag="rb")
            for g in range(4):
                p0 = 32 * g
                nc.sync.dma_start(ls_a[p0 : p0 + K, :], la[:])
                nc.sync.dma_start(rs_a[p0 : p0 + K, :], ra[:])
                nc.sync.dma_start(ls_b[p0 : p0 + K, :], lb[:])
                nc.sync.dma_start(rs_b[p0 : p0 + K, :], rb[:])

            out_s = resp.tile([128, 2 * NT], F32)

            for p, (lt, rt) in enumerate([(ls_a, rs_a), (ls_b, rs_b)]):
                for t in range(NT):
                    col = p * NT + t
                    direct = (t % DIRECT_EVERY) == (DIRECT_EVERY - 1)
                    if direct:
                        part = partp.tile([128, NJ], F32)
                        stile = None
                    else:
                        stile = stg.tile([128, M], F16, tag="stage")
                        part = None
                    for j in range(NJ):
                        ps = psum.tile([128, MCHUNK], F32)
                        for g in range(4):
                            p0 = 32 * g
                            c0 = j * MCHUNK + g * 512
                            nc.tensor.matmul(
                                ps[:, g * 512 : (g + 1) * 512],
                                lt[p0 : p0 + K, t * 128 : (t + 1) * 128],
                                rt[p0 : p0 + K, c0 : c0 + 512],
                                tile_position=(p0, 0),
                            )
                        if direct:
                            nc.vector.tensor_reduce(
                                part[:, j : j + 1],
                                ps[:],
                                axis=mybir.AxisListType.X,
                                op=mybir.AluOpType.min,
                            )
                        else:
                            nc.scalar.activation(
                                stile[:, j * MCHUNK : (j + 1) * MCHUNK],
                                ps[:],
                                mybir.ActivationFunctionType.Relu,
                            )
                    if direct:
                        nc.vector.tensor_reduce(
                            out_s[:, col : col + 1],
                            part[:],
                            axis=mybir.AxisListType.X,
                            op=mybir.AluOpType.min,
                        )
                    else:
                        # fp16 TT-min fold tree: 8192 -> 512, then reduce
                        fold = junkp.tile([128, M // 2], F16, tag="fold")
                        w = M // 2
                        src = stile
                        for _ in range(4):
                            nc.vector.tensor_tensor(
                                fold[:, :w],
                                src[:, :w],
                                src[:, w : 2 * w],
                                op=mybir.AluOpType.min,
                            )
                            src = fold
                            w //= 2
                        nc.vector.tensor_reduce(
                            out_s[:, col : col + 1],
                            fold[:, : 2 * w],
                            axis=mybir.AxisListType.X,
                            op=mybir.AluOpType.min,
                        )
            nc.sync.dma_start(out[:], out_s[:])
    nc.compile()
    return nc


def _split16(v):
    h = v.astype(np.float16)
    l = (v - h.astype(np.float32)).astype(np.float16)
    return h, l


def _lhs_aug(p):
    # p: [n, 3] fp32 -> [13, n] fp16 hi/lo split; pairs with _rhs_aug rows so
    # that sum_k lhs[k,n]*rhs[k,m] = ||x||^2 + ||y||^2 - 2 x.y to ~fp32 accuracy
    p2 = np.sum(p * p, axis=-1)
    c = p.T
    ch, cl = _split16(c)
    p2h, p2l = _split16(p2)
    ones = np.ones_like(p2, dtype=np.float16)
    rows = [
        -2 * ch[0], -2 * ch[1], -2 * ch[2],  # * y_h
        -2 * ch[0], -2 * ch[1], -2 * ch[2],  # * y_l
        -2 * cl[0], -2 * cl[1], -2 * cl[2],  # * y_h
        p2h, p2l, ones, ones,
    ]
    return np.ascontiguousarray(np.stack([r.astype(np.float16) for r in rows], 0))


def _rhs_aug(p):
    p2 = np.sum(p * p, axis=-1)
    c = p.T
    ch, cl = _split16(c)
    p2h, p2l = _split16(p2)
    ones = np.ones_like(p2, dtype=np.float16)
    rows = [
        ch[0], ch[1], ch[2],
        cl[0], cl[1], cl[2],
        ch[0], ch[1], ch[2],
        ones, ones, p2h, p2l,
    ]
    return np.ascontiguousarray(np.stack([r.astype(np.float16) for r in rows], 0))


def _run(x, y, trace=False):
    if "nc" not in _NC_CACHE:
        _NC_CACHE["nc"] = build_bass()
    nc = _NC_CACHE["nc"]

    in_maps = []
    for c in range(N_CORES):
        b, h = divmod(c, 2)
        xs = x[b, h * HALF : (h + 1) * HALF]
        ys = y[b, h * HALF : (h + 1) * HALF]
        in_maps.append(
            {
                "la": _lhs_aug(xs),
                "ra": _rhs_aug(y[b]),
                "lb": _lhs_aug(ys),
                "rb": _rhs_aug(x[b]),
            }
        )
    return run_bass_kernel_spmd(nc, in_maps, list(range(N_CORES)), trace=trace)


def kernel(x, y, bidirectional):
    x = np.asarray(x, dtype=np.float32)
    y = np.asarray(y, dtype=np.float32)
    bidir = int(np.asarray(bidirectional))

    res = _run(x, y).results

    term1 = np.zeros(B, dtype=np.float64)
    term2 = np.zeros(B, dtype=np.float64)
    for c in range(N_CORES):
        b, h = divmod(c, 2)
        o = np.asarray(res[c]["out"])
        rma = o[:, :NT].T.reshape(-1)  # d2 row mins, x-half -> over all y
        rmb = o[:, NT:].T.reshape(-1)  # d2 row mins, y-half -> over all x
        term1[b] += np.sqrt(np.maximum(rma, 0.0)).mean() / 2.0
        term2[b] += np.sqrt(np.maximum(rmb, 0.0)).mean() / 2.0

    loss = term1.mean()
    if bidir:
        loss = loss + term2.mean()
    return np.float32(loss)


# revision 13
# speedup vs baseline: 1.7732x; 1.1206x over previous
"""Chamfer distance kernel for Trainium2 (8 NeuronCores).

Problem: x, y ~ (B=4, N=8192, 3) fp32. loss = mean_b[ mean_n min_m dist + mean_m min_n dist ].

Strategy:
  - d2[n,m] = ||x||^2 + ||y||^2 - 2 x.y as ONE augmented matmul: the fp32
    values are split hi/lo into fp16 (exact products in PSUM fp32), giving
    K=13 fp16 contraction rows -> 4x faster PE than fp32, ~1e-7 accuracy.
  - K=13 <= 32, so 4 independent matmuls are packed into the PE array via
    tile_position row-groups (base partitions 0/32/64/96) -> ~4x concurrency.
  - min over the free axis; sqrt/means on host (monotone => sqrt after min).
  - Two consumer lanes, balanced so DVE and ACT both stay busy:
      direct blocks: DVE reduce_min straight from PSUM (1 elem/lane/cyc)
      ACT blocks:    ScalarE Relu-cast-copies PSUM->SBUF fp16, then DVE
                     tensor_tensor min folds at 2x (fp16 2X_1P) + one reduce
    (tensor_tensor_reduce would be 1 instr/block but crashes this runtime)
  - 8 cores = (batch b, half h): each core does both directions for its half
    of the rows against the full opposite set => no cross-core reduction.
"""

import sys

sys.path.insert(0, "/opt/trn_rl_repo")

import numpy as np

import concourse.bass as bass
import concourse.bacc as bacc
import concourse.mybir as mybir
from concourse import tile
from concourse.bass_utils import run_bass_kernel_spmd

N_CORES = 8
B, N, M, D = 4, 8192, 8192, 3
HALF = N // 2  # rows per core per direction
NT = HALF // 128  # 32 n-tiles per pass
MCHUNK = 2048  # psum tile free size (4 banks)
NJ = M // MCHUNK  # 4 chunks per block
K = 13  # augmented fp16 hi/lo contraction rows
DIRECT_EVERY = int(__import__("os").environ.get("CHAMFER_DIRECT_EVERY", "5"))
STAGE_BUFS = int(__import__("os").environ.get("CHAMFER_STAGE_BUFS", "3"))
BIG = 60000.0  # > max possible d2, fits fp16
F32 = mybir.dt.float32
F16 = mybir.dt.float16

_NC_CACHE = {}


def build_bass():
    nc = bacc.Bacc(
        "TRN2", target_bir_lowering=False, debug=False, num_devices=N_CORES
    )
    la = nc.dram_tensor("la", [K, HALF], F16, kind="ExternalInput")
    ra = nc.dram_tensor("ra", [K, M], F16, kind="ExternalInput")
    lb = nc.dram_tensor("lb", [K, HALF], F16, kind="ExternalInput")
    rb = nc.dram_tensor("rb", [K, M], F16, kind="ExternalInput")
    out = nc.dram_tensor("out", [128, 2 * NT], F32, kind="ExternalOutput")

    with tile.TileContext(nc) as tc:
        with (
            tc.tile_pool(name="inp", bufs=1) as inp,
            tc.tile_pool(name="psum", bufs=2, space="PSUM") as psum,
            tc.tile_pool(name="stg", bufs=STAGE_BUFS) as stg,
            tc.tile_pool(name="junkp", bufs=2) as junkp,
            tc.tile_pool(name="part", bufs=4) as partp,
            tc.tile_pool(name="res", bufs=1) as resp,
        ):
            # inputs replicated at base partitions 0/32/64/96 for row-group
            # packed matmuls (4 concurrent MMs in the PE array)
            ls_a = inp.tile([128, HALF], F16, tag="la")
            rs_a = inp.tile([128, M], F16, tag="ra")
            ls_b = inp.tile([128, HALF], F16, tag="lb")
            rs_b = inp.tile([128, M], F16, tag="rb")
            for g in range(4):
                p0 = 32 * g
                nc.sync.dma_start(ls_a[p0 : p0 + K, :], la[:])
                nc.sync.dma_start(rs_a[p0 : p0 + K, :], ra[:])
                nc.sync.dma_start(ls_b[p0 : p0 + K, :], lb[:])
                nc.sync.dma_start(rs_b[p0 : p0 + K, :], rb[:])

            out_s = resp.tile([128, 2 * NT], F32)

            for p, (lt, rt) in enumerate([(ls_a, rs_a), (ls_b, rs_b)]):
                for t in range(NT):
                    col = p * NT + t
                    direct = (t % DIRECT_EVERY) == (DIRECT_EVERY - 1)
                    if direct:
                        part = partp.tile([128, NJ], F32)
                        stile = None
                    else:
                        stile = stg.tile([128, M], F16, tag="stage")
                        part = None
                    for j in range(NJ):
                        ps = psum.tile([128, MCHUNK], F32)
                        for g in range(4):
                            p0 = 32 * g
                            c0 = j * MCHUNK + g * 512
                            nc.tensor.matmul(
                                ps[:, g * 512 : (g + 1) * 512],
                                lt[p0 : p0 + K, t * 128 : (t + 1) * 128],
                                rt[p0 : p0 + K, c0 : c0 + 512],
                                tile_position=(p0, 0),
                            )
                        if direct:
                            nc.vector.tensor_reduce(
                                part[:, j : j + 1],
                                ps[:],
                                axis=mybir.AxisListType.X,
                                op=mybir.AluOpType.min,
                            )
                        else:
                            nc.scalar.activation(
                                stile[:, j * MCHUNK : (j + 1) * MCHUNK],
                                ps[:],
                                mybir.ActivationFunctionType.Relu,
                            )
                    if direct:
                        nc.vector.tensor_reduce(
                            out_s[:, col : col + 1],
                            part[:],
                            axis=mybir.AxisListType.X,
                            op=mybir.AluOpType.min,
                        )
                    else:
                        # fp16 TT-min fold tree: 8192 -> 512, then reduce
                        fold = junkp.tile([128, M // 2], F16, tag="fold")
                        w = M // 2
                        src = stile
                        for _ in range(4):
                            nc.vector.tensor_tensor(
                                fold[:, :w],
                                src[:, :w],
                                src[:, w : 2 * w],
                                op=mybir.AluOpType.min,
                            )
                            src = fold
                            w //= 2
                        nc.vector.tensor_reduce(
                            out_s[:, col : col + 1],
                            fold[:, : 2 * w],
                            axis=mybir.AxisListType.X,
                            op=mybir.AluOpType.min,
                        )
            nc.sync.dma_start(out[:], out_s[:])
    nc.compile()
    return nc


def _split16(v):
    h = v.astype(np.float16)
    l = (v - h.astype(np.float32)).astype(np.float16)
    return h, l


def _lhs_aug(p):
    # p: [n, 3] fp32 -> [13, n] fp16 hi/lo split; pairs with _rhs_aug rows so
    # that sum_k lhs[k,n]*rhs[k,m] = ||x||^2 + ||y||^2 - 2 x.y to ~fp32 accuracy
    p2 = np.sum(p * p, axis=-1)
    c = p.T
    ch, cl = _split16(c)
    p2h, p2l = _split16(p2)
    ones = np.ones_like(p2, dtype=np.float16)
    rows = [
        -2 * ch[0], -2 * ch[1], -2 * ch[2],  # * y_h
        -2 * ch[0], -2 * ch[1], -2 * ch[2],  # * y_l
        -2 * cl[0], -2 * cl[1], -2 * cl[2],  # * y_h
        p2h, p2l, ones, ones,
    ]
    return np.ascontiguousarray(np.stack([r.astype(np.float16) for r in rows], 0))


def _rhs_aug(p):
    p2 = np.sum(p * p, axis=-1)
    c = p.T
    ch, cl = _split16(c)
    p2h, p2l = _split16(p2)
    ones = np.ones_like(p2, dtype=np.float16)
    rows = [
        ch[0], ch[1], ch[2],
        cl[0], cl[1], cl[2],
        ch[0], ch[1], ch[2],
        ones, ones, p2h, p2l,
    ]
    return np.ascontiguousarray(np.stack([r.astype(np.float16) for r in rows], 0))


def _run(x, y, trace=False):
    if "nc" not in _NC_CACHE:
        _NC_CACHE["nc"] = build_bass()
    nc = _NC_CACHE["nc"]

    in_maps = []
    for c in range(N_CORES):
        b, h = divmod(c, 2)
        xs = x[b, h * HALF : (h + 1) * HALF]
        ys = y[b, h * HALF : (h + 1) * HALF]
        in_maps.append(
            {
                "la": _lhs_aug(xs),
                "ra": _rhs_aug(y[b]),
                "lb": _lhs_aug(ys),
                "rb": _rhs_aug(x[b]),
            }
        )
    return run_bass_kernel_spmd(nc, in_maps, list(range(N_CORES)), trace=trace)


def kernel(x, y, bidirectional):
    x = np.asarray(x, dtype=np.float32)
    y = np.asarray(y, dtype=np.float32)
    bidir = int(np.asarray(bidirectional))

    res = _run(x, y).results

    term1 = np.zeros(B, dtype=np.float64)
    term2 = np.zeros(B, dtype=np.float64)
    for c in range(N_CORES):
        b, h = divmod(c, 2)
        o = np.asarray(res[c]["out"])
        rma = o[:, :NT].T.reshape(-1)  # d2 row mins, x-half -> over all y
        rmb = o[:, NT:].T.reshape(-1)  # d2 row mins, y-half -> over all x
        term1[b] += np.sqrt(np.maximum(rma, 0.0)).mean() / 2.0
        term2[b] += np.sqrt(np.maximum(rmb, 0.0)).mean() / 2.0

    loss = term1.mean()
    if bidir:
        loss = loss + term2.mean()
    return np.float32(loss)
